# revision 55
# baseline (speedup 1.0000x reference)
"""Trainium2 Bass kernel for nn_MultiHeadDotProductAttention (b=4, L=2048,
d_model=1024, 16 heads x 64 head_dim, additive attention bias, softmax).

Sharding: 8 cores = 2 batch-groups (2 batches each) x 4 head-groups (4 heads
each). Each core computes, for its 2 batches and 4 heads, the full attention
pipeline and an output-projection PARTIAL (summed over its 4 heads); the host
sums the 4 head-group partials per batch and adds the output bias.

Device layout ("T layout"): sequence-length on the free dim, feature dims on
partitions, so no on-device transposes are needed:
  qT,kT: [hd, l]    from  out = wq^T @ xT  (xT transposed on host)
  logitsT[lk, lq] = kT-slices^T @ qT (K=64, two heads row-packed via
                    tile_position into one [128,1024] 2-bank PSUM tile)
  softmax: the additive bias is applied MULTIPLICATIVELY: the host streams
           expb = exp(bias) as bf16 and the device computes
           attn = exp(logits) * expb  (one wide ACT exp from PSUM + one
           bf16 2x-mode DVE multiply). Denominators come for free from an
           all-ones column appended to V in the AV matmul; normalization is
           reciprocal_approx_fast + ones-matmul partition-broadcast + DVE mul.
  out = ctxT^T @ wo with ctxT [hd, lq] directly produced by AV.

Algebraic simplifications vs the reference:
  - bk is dropped entirely: q.(k+bk) adds a per-(b,h,lq) constant to every
    logit in the softmax row, which cancels exactly in softmax.
  - bv is dropped on-device: its contribution is ctx += bv per head, so
    out += sum_h bv_h @ wo_h -- a constant [D] vector added on the host.
  - the 1/sqrt(head_dim) query scale is folded into wq/bq on the host.

Schedule (final): P1 projections stream with interleaved per-k-chunk
weight+x DMAs (first matmul ~1us in, after PE warm-up matmuls and an early
exp-table preload); the last group's v-projection is deferred into P2's
first half-blocks, whose PE would otherwise idle against the ACT exp.
P2 runs n-major half-blocks with logits emitted FIRST each step (earliest
exp start), then the 5-step-lagged AV accumulation and one paced P3 unit
(arrival-rate pacing: 1 per 4 steps), so the PE has surplus ready work in
every stretch. PSUM banks: logits 2x2-bank ring, av 2x1 single-buffered
(safe because of the AV lag), po 2x1 dedicated. The normalize is split:
PSUM-freeing copy + reciprocal at the boundary, gpsimd broadcasts with a
4-step deadline, in-SBUF scale at i==4, after which that (n,b)'s P3 units
release. The tail normalize broadcasts via a K=1 ones-matmul instead of
gpsimd, and tail P3 units alternate ACT/DVE drains. expb streams from a
host-pretiled contiguous layout for full-size DMA packets.
"""

import numpy as np
from contextlib import ExitStack

import ml_dtypes

import concourse.bass as bass
import concourse.mybir as mybir
import concourse.tile as tile
from concourse import bacc
from concourse import bass_utils

F32 = mybir.dt.float32
F32R = mybir.dt.float32r
BF16 = mybir.dt.bfloat16
AF = mybir.ActivationFunctionType

# ---- problem constants (hardcoded per contract) ----
B, L, D = 4, 2048, 1024
H, DH = 16, 64
NB = 2          # batch groups (batches per core = B // NB = 2)
NH = 4          # head groups  (heads per core = H // NH = 4)
BPC = B // NB   # 2 batches per core
HPC = H // NH   # 4 heads per core
PAIRS = HPC // 2
KSUB = D // 128          # 8 contraction subtiles for projections
NQ = 4                   # lq chunks of 512 for attention
NI = 16                  # lk chunks of 128
HD = HPC * DH            # 256 local head dims
HDC = HD // 128          # 2 local hd chunks (= PAIRS)

# P3 partial-output dtype streamed back to the host
OUT_DT = "bf16"

_CACHED = {}


def _build_bass():
    nc = bacc.Bacc("TRN2", target_bir_lowering=False, debug=False, num_devices=8)

    out_dt = F32 if OUT_DT == "f32" else BF16

    # ---- DRAM I/O (per core) ----
    xq_d = nc.dram_tensor("xq_t", [BPC, D, L], BF16, kind="ExternalInput")
    xk_d = nc.dram_tensor("xk_t", [BPC, D, L], BF16, kind="ExternalInput")
    # expb pre-tiled on host: [pair, n, i, part(lk 128), hl, lq 512]
    expb_d = nc.dram_tensor(
        "expb_t", [PAIRS, NQ, NI, 128, 2, 512], BF16, kind="ExternalInput"
    )
    wq_d = nc.dram_tensor("wq", [D, HD], BF16, kind="ExternalInput")
    wk_d = nc.dram_tensor("wk", [D, HD], BF16, kind="ExternalInput")
    wv_d = nc.dram_tensor("wv", [D, HD], BF16, kind="ExternalInput")
    wo_d = nc.dram_tensor("wo", [HD, D], BF16, kind="ExternalInput")
    bq_d = nc.dram_tensor("bq", [HD], F32, kind="ExternalInput")
    out_d = nc.dram_tensor("out_part", [BPC, L, D], out_dt, kind="ExternalOutput")

    with tile.TileContext(nc) as tc, ExitStack() as top:
        # ---- persistent SBUF ----
        pers = top.enter_context(tc.tile_pool(name="pers", bufs=1))
        qT = pers.tile([128, HDC, BPC, L], BF16)
        kT = pers.tile([128, HDC, BPC, L], BF16)
        v = pers.tile([128, NI, BPC, HPC, DH + 1], BF16)
        ctxT = pers.tile([128, HDC, BPC, L], BF16)
        wo_s = pers.tile([128, HDC, D], BF16)
        bq_s = pers.tile([128, HDC], F32)
        # wv + the last group's x stay alive into P2: that group's
        # v-projection is deferred into P2's first half-blocks, which are
        # otherwise ACT-bound with an idling PE
        wv_s = pers.tile([128, KSUB, HD], BF16)
        xdef = pers.tile([128, KSUB, 1024], BF16)
        ones_f32 = pers.tile([128, 128], F32)
        nc.vector.memset(ones_f32[:], 1.0)
        # preload the exp activation table while nothing depends on ACT: the
        # first real exp would otherwise eat the ~2.7us ACT_TABLE_LOAD at the
        # P1->P2 transition
        warm_act = pers.tile([1, 2], BF16)
        nc.scalar.activation(warm_act[:], ones_f32[0:1, 0:2], AF.Exp)
        # PE warm-up: the first ~6us of the kernel are DMA-latency-bound with
        # an idle PE; streaming junk matmuls ramps the PE out of its low
        # p-state so the first real projections run at full clock
        warm_w = pers.tile([128, 512], BF16)
        nc.vector.memset(warm_w[:], 0.0)
        # softmax-denominator column of v (column 0 is all-ones, so the
        # denominator row lands on PSUM partition 0 where the DVE reciprocal
        # and the gpsimd partition-broadcast can reach it directly)
        nc.vector.tensor_copy(
            v[:, :, :, :, 0],
            ones_f32[:, 0:NI * BPC * HPC].rearrange(
                "p (a b c) -> p a b c", a=NI, b=BPC
            ),
        )

        # ---- P1: projections (weight-stationary, N=512) ----
        with ExitStack() as p1:
            wpool = p1.enter_context(tc.tile_pool(name="wqkv", bufs=1))
            wq_s = wpool.tile([128, KSUB, HD], BF16)
            wk_s = wpool.tile([128, KSUB, HD], BF16)

            xpool = p1.enter_context(tc.tile_pool(name="xs", bufs=2))
            pp = p1.enter_context(tc.tile_pool(name="pqk", bufs=1, space="PSUM"))
            psv = p1.enter_context(tc.tile_pool(name="psv", bufs=3, space="PSUM"))

            with tc.tile_pool(name="warm", bufs=1, space="PSUM") as wrm:
                wt = wrm.tile([128, 512], F32)
                for wi in range(16):
                    nc.tensor.matmul(
                        wt[:], warm_w[:, 0:128], warm_w[:],
                        start=(wi == 0), stop=(wi == 15),
                    )

            groups = [(b, h2) for b in range(BPC) for h2 in range(2)]

            # group-0 x tiles up front so their chunk DMAs interleave with
            # the weight chunk DMAs: the first matmul needs only wq[k=0] +
            # xq0[k=0] (~320KB), so the PE starts ~1us in instead of ~15us.
            wqr = wq_d.rearrange("(k p) n -> p k n", p=128)
            wkr = wk_d.rearrange("(k p) n -> p k n", p=128)
            wvr = wv_d.rearrange("(k p) n -> p k n", p=128)
            g0_xq = xpool.tile([128, KSUB, 1024], BF16, tag="xq")
            g0_xk = xpool.tile([128, KSUB, 1024], BF16, tag="xk")
            b0, h20 = groups[0]
            xqr0 = xq_d[b0].rearrange("(k p) l -> p k l", p=128)
            xkr0 = xk_d[b0].rearrange("(k p) l -> p k l", p=128)
            hsl0 = slice(h20 * 1024, (h20 + 1) * 1024)
            # k=0 in half-chunks so the very first matmul (needs only
            # wq[k0, m0] + xq0[k0, c0]) starts as early as possible
            nc.sync.dma_start(wq_s[:, 0, 0:128], wqr[:, 0, 0:128])
            nc.sync.dma_start(g0_xq[:, 0, 0:512], xqr0[:, 0, hsl0][:, 0:512])
            nc.sync.dma_start(wq_s[:, 0, 128:HD], wqr[:, 0, 128:HD])
            nc.sync.dma_start(g0_xq[:, 0, 512:1024], xqr0[:, 0, hsl0][:, 512:1024])
            nc.sync.dma_start(wk_s[:, 0, :], wkr[:, 0, :])
            nc.sync.dma_start(g0_xk[:, 0, :], xkr0[:, 0, hsl0])
            nc.sync.dma_start(wv_s[:, 0, :], wvr[:, 0, :])
            for k in range(1, KSUB):
                nc.sync.dma_start(wq_s[:, k, :], wqr[:, k, :])
                nc.sync.dma_start(g0_xq[:, k, :], xqr0[:, k, hsl0])
                nc.sync.dma_start(wk_s[:, k, :], wkr[:, k, :])
                nc.sync.dma_start(g0_xk[:, k, :], xkr0[:, k, hsl0])
                nc.sync.dma_start(wv_s[:, k, :], wvr[:, k, :])
            nc.sync.dma_start(bq_s[:], bq_d.rearrange("(c p) -> p c", p=128))
            nc.sync.dma_start(wo_s[:], wo_d.rearrange("(c p) n -> p c n", p=128))

            def qk_gen(xq_t, xk_t, b, h2):
                # q then k: 4-bank groups accumulating over KSUB with a
                # stationary weight slice serving both 512-wide chunks; each
                # bank drains (ACT for q with bias, DVE for k) right after
                # its last accumulation matmul.
                for which in range(2):
                    w_s = wq_s if which == 0 else wk_s
                    x_t = xq_t if which == 0 else xk_t
                    ps = {}
                    for m in range(HDC):
                        for c in range(2):
                            ps[m, c] = pp.tile(
                                [128, 512], F32, tag=f"p{m}{c}",
                                name=f"ps{m}{c}",
                            )
                    for k in range(KSUB):
                        for m in range(HDC):
                            msl = slice(m * 128, (m + 1) * 128)
                            for c in range(2):
                                csl = slice(c * 512, (c + 1) * 512)
                                nc.tensor.matmul(
                                    ps[m, c][:], w_s[:, k, msl], x_t[:, k, csl],
                                    start=(k == 0), stop=(k == KSUB - 1),
                                )
                                if k == KSUB - 1:
                                    osl = slice(
                                        h2 * 1024 + c * 512,
                                        h2 * 1024 + (c + 1) * 512,
                                    )
                                    if which == 0:
                                        nc.scalar.activation(
                                            qT[:, m, b, osl], ps[m, c][:],
                                            AF.Identity, bias=bq_s[:, m:m + 1],
                                        )
                                    else:
                                        nc.vector.tensor_copy(
                                            kT[:, m, b, osl], ps[m, c][:]
                                        )
                                yield

            def v_gen(xk_t, b, h2):
                # v: out[lk-sub(128), hd(256)] = xT-slices^T @ wv
                for s in range(8):
                    si = h2 * 8 + s
                    pv = psv.tile([128, HD], F32, tag="pv", name="pv")
                    for k in range(KSUB):
                        nc.tensor.matmul(
                            pv[:], xk_t[:, k, s * 128:(s + 1) * 128],
                            wv_s[:, k, :],
                            start=(k == 0), stop=(k == KSUB - 1),
                        )
                        if k == KSUB - 1:
                            nc.vector.tensor_copy(
                                v[:, si, b, :, 1:DH + 1],
                                pv[:].rearrange("p (h d) -> p h d", h=HPC),
                            )
                        yield

            from itertools import zip_longest

            for gi, (b, h2) in enumerate(groups):
                hsl = slice(h2 * 1024, (h2 + 1) * 1024)
                last = gi == len(groups) - 1
                if gi == 0:
                    xq_t, xk_t = g0_xq, g0_xk
                else:
                    xqr = xq_d[b].rearrange("(k p) l -> p k l", p=128)
                    xkr = xk_d[b].rearrange("(k p) l -> p k l", p=128)
                    xq_t = xpool.tile([128, KSUB, 1024], BF16, tag="xq")
                    xk_t = xdef if last else xpool.tile(
                        [128, KSUB, 1024], BF16, tag="xk"
                    )
                    # per-k-subtile DMAs: the k=0 matmuls start as soon as
                    # the first chunk lands instead of waiting for 4MB
                    for k in range(KSUB):
                        nc.sync.dma_start(xq_t[:, k, :], xqr[:, k, hsl])
                        nc.sync.dma_start(xk_t[:, k, :], xkr[:, k, hsl])

                # interleave v matmuls 1:1 between q/k matmuls so the
                # v LDWEIGHTS loads hide under the longer N=512 matmuls.
                # The LAST group's v is deferred into P2's first half-blocks.
                for _ in zip_longest(
                    qk_gen(xq_t, xk_t, b, h2),
                    iter(()) if last else v_gen(xk_t, b, h2),
                ):
                    pass

        # ---- P2: attention (+ P3 output projection interleaved) ----
        # n-major half-blocks, 2-step AV lag. Per i-step the PE does
        # AV(i-2) + logits(i); the previous half-block's last two AV
        # accumulations, its normalize, and the per-(n,b) P3 release all
        # happen at i==0 of the following half-block.
        with ExitStack() as p2:
            ebpool = p2.enter_context(tc.tile_pool(name="ebb", bufs=2))
            epool = p2.enter_context(tc.tile_pool(name="expb", bufs=8))
            apool = p2.enter_context(tc.tile_pool(name="attnb", bufs=9))
            rpool = p2.enter_context(tc.tile_pool(name="recip", bufs=2))
            scpool = p2.enter_context(tc.tile_pool(name="scsh", bufs=2))
            opool = p2.enter_context(tc.tile_pool(name="outb", bufs=4))
            pslg = p2.enter_context(tc.tile_pool(name="pslg", bufs=2, space="PSUM"))
            psav = p2.enter_context(tc.tile_pool(name="psav", bufs=1, space="PSUM"))
            # the last two PSUM banks first serve the deferred v-projection
            # (psv2), then are handed over to the P3 drain pool (pspo)
            pools = {"psv2": tc.alloc_tile_pool(name="psv2", bufs=2, space="PSUM")}

            def emit_deferred_v():
                s = deferred_v.pop(0)
                si = 8 + s
                pv = pools["psv2"].tile([128, HD], F32, tag="pv2", name="pv2")
                for k in range(KSUB):
                    nc.tensor.matmul(
                        pv[:], xdef[:, k, s * 128:(s + 1) * 128],
                        wv_s[:, k, :],
                        start=(k == 0), stop=(k == KSUB - 1),
                    )
                nc.vector.tensor_copy(
                    v[:, si, 1, :, 1:DH + 1],
                    pv[:].rearrange("p (h d) -> p h d", h=HPC),
                )
                if not deferred_v:
                    pools["psv2"].release()
                    pools["pspo"] = p2.enter_context(
                        tc.tile_pool(name="pspo", bufs=2, space="PSUM")
                    )

            def emit_norm_early(p, n, b, av):
                # Stage 1 of normalize, at the half-block boundary: copy the
                # UNNORMALIZED ctx rows out of PSUM (frees the single-buffered
                # av banks quickly) and take the denominator reciprocal.
                # Both run on the ACT engine: its queue is short at the
                # boundary while the DVE is still chewing the last attn
                # multiplies, so the av banks free ~1us earlier. The slow
                # gpsimd partition-broadcasts get a ~4-step deadline.
                staged = []
                for hl in range(2):
                    sc = scpool.tile([65, 512], BF16, tag="sc", name="sc")
                    nc.vector.tensor_copy(sc[0:65, :], av[hl][0:DH + 1, :])
                    rcp = rpool.tile([1, 512], F32, tag="rcp", name="rcp")
                    nc.vector.reciprocal_approx_fast(rcp[:], av[hl][0:1, :])
                    rep = rpool.tile([65, 512], F32, tag="rep", name="rep")
                    nc.gpsimd.partition_broadcast(rep[:], rcp[0:1, :])
                    staged.append((sc, rep))
                return staged

            def emit_norm_mul(p, n, b, staged):
                # Stage 2: scale the staged ctx rows by the broadcast
                # reciprocals and ship them into their ctxT partition slot.
                # After this lands the (n, b) slice is P3-ready.
                nsl = slice(n * 512, (n + 1) * 512)
                for hl in range(2):
                    sc, rep = staged[hl]
                    scn = scpool.tile([65, 512], BF16, tag="scn", name="scn")
                    nc.vector.tensor_mul(scn[0:65, :], sc[0:65, :], rep[0:65, :])
                    nc.sync.dma_start(
                        ctxT[hl * 64:(hl + 1) * 64, p, b, nsl], scn[1:65, :]
                    )

            def emit_p3_unit(b, m, nn, act_drain=False, tail=False):
                msl = slice(m * 128, (m + 1) * 128)
                osl = slice(nn * 512, (nn + 1) * 512)
                po = pools["pspo"].tile([128, 512], F32, tag="po", name="po")
                for kc in range(HDC):
                    nc.tensor.matmul(
                        po[:], ctxT[:, kc, b, msl], wo_s[:, kc, osl],
                        start=(kc == 0), stop=(kc == HDC - 1),
                    )
                ot = opool.tile([128, 512], out_dt, tag="ot", name="ot")
                if act_drain:
                    # tail units drain on the ACT engine, which is idle once
                    # the last exp has issued
                    nc.scalar.activation(ot[:], po[:], AF.Identity)
                else:
                    nc.vector.tensor_copy(ot[:], po[:])
                nc.sync.dma_start(out_d[b, msl, osl], ot[:])

            def emit_eb_dmas(p, n, i, eb_store):
                ebt = ebpool.tile(
                    [128, 2, 512], BF16, tag=f"eb{i}", name=f"eb{i}"
                )
                nc.sync.dma_start(ebt[:], expb_d[p, n, i])
                eb_store[i] = ebt

            halfblocks = [
                (p, n, b)
                for n in range(NQ)
                for p in range(PAIRS)
                for b in range(BPC)
            ]
            nblocks = [(p, n) for n in range(NQ) for p in range(PAIRS)]
            eb_cur, eb_nxt = {}, {}
            for i in range(NI):
                emit_eb_dmas(nblocks[0][0], nblocks[0][1], i, eb_cur)

            pending_norm = None   # (p, n, b, av) awaiting stage-1 emission
            pending_mul = None    # (p, n, b, reps) awaiting stage-2 emission
            pending_av = []       # (av, i, b, p, at) -- AV matmuls lag 3 steps
            AV_LAG = 4
            p3_queue = []         # (b, m, nn) output-projection units
            deferred_v = list(range(8))   # s-columns of the (b1,h2=1) v-proj

            def emit_pending_av(drain=False, force=0):
                npop = 0
                while pending_av and (
                    drain or len(pending_av) > AV_LAG or npop < force
                ):
                    npop += 1
                    pav, pi, pb, pp_, pat = pending_av.pop(0)
                    for hl in range(2):
                        nc.tensor.matmul(
                            pav[hl][0:DH + 1, :],
                            v[:, pi, pb, 2 * pp_ + hl, :],
                            pat[:, hl * 512:(hl + 1) * 512],
                            start=(pi == 0), stop=(pi == NI - 1),
                        )

            def release_p3(pn):
                # after the LAST pair's normalize for (n, b), that (n, b)'s
                # 8 output-projection units are computable
                pn_p, pn_n, pn_b = pn[0], pn[1], pn[2]
                if pn_p == PAIRS - 1:
                    p3_queue.extend(
                        (pn_b, m, nn)
                        for m in range(4 * pn_n, 4 * pn_n + 4)
                        for nn in range(D // 512)
                    )

            for hbi, (p, n, b) in enumerate(halfblocks):
                if True:
                    nsl = slice(n * 512, (n + 1) * 512)
                    av = {}
                    for hl in range(2):
                        av[hl] = psav.tile(
                            [128, 512], F32, tag=f"av{hl}", name=f"av{hl}"
                        )
                    for i in range(NI):
                        isl = slice(i * 128, (i + 1) * 128)
                        # steady state: the lagged AV (whose attn tile is
                        # ready) and any P3 / deferred-v unit are emitted
                        # BEFORE this step's logits, so a logits matmul
                        # waiting on its PSUM ring can't head-of-line-block
                        # work whose inputs are already available.
                        # logits FIRST: with the deep AV lag the AV/P3
                        # inputs are long-ready, so they never head-of-line
                        # block; issuing LG early lets the exp start ~0.75us
                        # sooner each step, decompressing the 2-deep lgp ring
                        lgp = pslg.tile([128, 1024], F32, tag="lgp", name="lgp")
                        for hl in range(2):
                            rsl = slice(hl * 64, (hl + 1) * 64)
                            nc.tensor.matmul(
                                lgp[:, hl * 512:(hl + 1) * 512],
                                kT[rsl, p, b, isl],
                                qT[rsl, p, b, nsl],
                                start=True, stop=True,
                                tile_position=(hl * 64, 0),
                            )
                        if i > 0:
                            emit_pending_av()
                            if deferred_v and i % 2 == 1:
                                emit_deferred_v()
                            # pace P3 at its arrival rate (8 units per 2
                            # half-blocks = 1 per 4 steps) so the PE has
                            # surplus work in EVERY stretch of the schedule
                            # and a standing backlog remains to cover the
                            # final normalize chain
                            if p3_queue and i >= 2 and (
                                i % 4 == 2
                                or (len(p3_queue) > 8 and i % 2 == 0)
                            ):
                                emit_p3_unit(*p3_queue.pop(0))
                        if i == 0:
                            emit_pending_av(drain=True)
                            if pending_norm is not None:
                                reps = emit_norm_early(*pending_norm)
                                pending_mul = pending_norm[:3] + (reps,)
                                pending_norm = None
                        if i == 4 and pending_mul is not None:
                            emit_norm_mul(*pending_mul)
                            release_p3(pending_mul)
                            pending_mul = None
                        et = epool.tile([128, 1024], BF16, tag="exp", name="et")
                        nc.scalar.activation(et[:], lgp[:], AF.Exp)
                        at = apool.tile([128, 1024], BF16, tag="attn", name="at")
                        nc.vector.tensor_mul(
                            at[:], et[:],
                            eb_cur[i].rearrange("p h n -> p (h n)"),
                        )
                        pending_av.append((av, i, b, p, at))
                        # prefetch next block's expb tiles (2 per step, b==0)
                        if b == 0 and hbi // 2 + 1 < len(nblocks):
                            np_, nn_ = nblocks[hbi // 2 + 1]
                            for j in (2 * i, 2 * i + 1):
                                if j < NI:
                                    emit_eb_dmas(np_, nn_, j, eb_nxt)
                    pending_norm = (p, n, b, av)
                    if b == BPC - 1:
                        eb_cur, eb_nxt = eb_nxt, {}

            # ---- tail drain ----
            emit_pending_av(drain=True)
            if pending_mul is not None:
                emit_norm_mul(*pending_mul)
                release_p3(pending_mul)
                pending_mul = None
            final_stage = None
            if pending_norm is not None:
                # final normalize, low-latency variant: skip the slow gpsimd
                # broadcasts; the reciprocal row is broadcast across
                # partitions by a K=1 ones-matmul into the (now dead) av
                # banks -- at the tail PSUM is free and the PE is cheap.
                fp, fn, fb, fav = pending_norm
                staged = []
                for hl in range(2):
                    sc = scpool.tile([65, 512], BF16, tag="sc", name="sc")
                    nc.vector.tensor_copy(sc[0:65, :], fav[hl][0:DH + 1, :])
                    rcp = rpool.tile([1, 512], F32, tag="rcp", name="rcp")
                    nc.vector.reciprocal_approx_fast(rcp[:], fav[hl][0:1, :])
                    staged.append((sc, rcp))
                pending_norm = None
                final_stage = (fp, fn, fb, staged)
            # leftover backlog units overlap the final normalize's DVE work
            head = [u for u in p3_queue[:8]]
            del p3_queue[:8]
            for idx, unit in enumerate(head):
                emit_p3_unit(*unit, act_drain=(idx % 2 == 0), tail=True)
            if final_stage is not None:
                fp, fn, fb, staged = final_stage
                nsl = slice(fn * 512, (fn + 1) * 512)
                reps_ps = []
                for hl in range(2):
                    rep_ps = psav.tile(
                        [128, 512], F32, tag=f"av{hl}", name=f"rep{hl}"
                    )
                    nc.tensor.matmul(
                        rep_ps[0:65, :], ones_f32[0:1, 0:65],
                        staged[hl][1][:], start=True, stop=True,
                    )
                    reps_ps.append(rep_ps)
                for hl in range(2):
                    scn = scpool.tile([65, 512], BF16, tag="scn", name="scn")
                    nc.vector.tensor_mul(
                        scn[0:65, :], staged[hl][0][0:65, :],
                        reps_ps[hl][0:65, :],
                    )
                    nc.sync.dma_start(
                        ctxT[hl * 64:(hl + 1) * 64, fp, fb, nsl], scn[1:65, :]
                    )
                release_p3((fp, fn, fb))
            for idx, unit in enumerate(p3_queue):
                emit_p3_unit(*unit, act_drain=(idx % 2 == 0), tail=True)

    nc.compile()
    return nc


def make_in_maps(inputs_q, inputs_kv, bias, wq, bq, wk, bk, wv, bv, wo, bo):
    inputs_q = np.asarray(inputs_q, np.float32)
    inputs_kv = np.asarray(inputs_kv, np.float32)
    bias = np.asarray(bias, np.float32)
    wq = np.asarray(wq, np.float32).reshape(D, H * DH)
    wv = np.asarray(wv, np.float32).reshape(D, H * DH)
    wk = np.asarray(wk, np.float32).reshape(D, H * DH)
    bq = np.asarray(bq, np.float32).reshape(H * DH)
    wo = np.asarray(wo, np.float32).reshape(H * DH, D)

    # fold the 1/sqrt(head_dim) query scaling into wq/bq
    s = 1.0 / np.sqrt(DH)
    wq = wq * s
    bq = bq * s

    # host-side layout marshalling for the chosen sharding
    xqT = np.ascontiguousarray(inputs_q.transpose(0, 2, 1)).astype(
        ml_dtypes.bfloat16
    )
    xkT = np.ascontiguousarray(inputs_kv.transpose(0, 2, 1)).astype(
        ml_dtypes.bfloat16
    )
    # multiplicative attention bias, pre-transposed AND pre-tiled into the
    # exact per-(pair, n, i) [128, 2, 512] device tiles so every expb DMA
    # is a single fully-contiguous 256KB transfer:
    #   expbT[h, lk, lq] -> [pair, n(lq/512), i(lk/128), 128, hl, 512]
    expbT = np.exp(bias[0].transpose(0, 2, 1)).astype(ml_dtypes.bfloat16)
    ebtiles = np.ascontiguousarray(
        expbT.reshape(H // 2, 2, NI, 128, NQ, 512).transpose(0, 4, 2, 3, 1, 5)
    )  # [8 pairs, n, i, 128, hl, 512]

    in_maps = []
    for bg in range(NB):
        bsl = slice(bg * BPC, (bg + 1) * BPC)
        for hg in range(NH):
            csl = slice(hg * HD, (hg + 1) * HD)
            psl = slice(hg * PAIRS, (hg + 1) * PAIRS)
            in_maps.append(
                {
                    "xq_t": xqT[bsl],
                    "xk_t": xkT[bsl],
                    "expb_t": np.ascontiguousarray(ebtiles[psl]),
                    "wq": np.ascontiguousarray(wq[:, csl]).astype(ml_dtypes.bfloat16),
                    "wk": np.ascontiguousarray(wk[:, csl]).astype(ml_dtypes.bfloat16),
                    "wv": np.ascontiguousarray(wv[:, csl]).astype(ml_dtypes.bfloat16),
                    "wo": np.ascontiguousarray(wo[csl, :]).astype(ml_dtypes.bfloat16),
                    "bq": np.ascontiguousarray(bq[csl]),
                }
            )
    return in_maps


def assemble(results, bv, wo, bo):
    """Sum per-head-group output partials; bv's contribution to the output is
    the constant vector sum_h bv_h @ wo_h, added here alongside bo."""
    bv = np.asarray(bv, np.float32).reshape(H * DH)
    wo = np.asarray(wo, np.float32).reshape(H * DH, D)
    bo = np.asarray(bo, np.float32)
    out = np.zeros((B, L, D), np.float32)
    for bg in range(NB):
        for hg in range(NH):
            out[bg * BPC:(bg + 1) * BPC] += np.asarray(
                results[bg * NH + hg]["out_part"], np.float32
            )
    out += bo + bv @ wo
    return out


def get_nc():
    if "nc" not in _CACHED:
        _CACHED["nc"] = _build_bass()
    return _CACHED["nc"]


def kernel(inputs_q, inputs_kv, bias, wq, bq, wk, bk, wv, bv, wo, bo):
    in_maps = make_in_maps(
        inputs_q, inputs_kv, bias, wq, bq, wk, bk, wv, bv, wo, bo
    )
    res = bass_utils.run_bass_kernel_spmd(
        get_nc(), in_maps, core_ids=list(range(8))
    )
    return assemble(res.results, bv, wo, bo)
